# revision 52
# baseline (speedup 1.0000x reference)
"""Paged decode attention (nn_Attention_5626407157951) on 8 Trainium2 cores.

Tensor-parallel over heads: each core owns 4 of 32 heads. The kernel is
HBM-bound (per-core DMA tops out ~250 GB/s regardless of queue count), so
everything is sized to minimize bytes: single-term bf16 matmuls everywhere
(vs hi/lo fp32 emulation: 3x less PE work and 33% less traffic), and the
v columns of W_pack in fp8*64 (v_new feeds one token of ~500; measured
total error 9e-3 vs the 2e-2 gate). Per core:
  qkv = hidden @ W_pack[:, own cols]      (bf16 / fp8, fp32 acc in PSUM)
  rotary(q, k) at pos=hist                (DVE, fp32; host-built cos/sin)
  scores_T[s, (h,p)] = K_cache^T q        (PE, K stationary, q moving 1 col)
  softmax without max-subtraction; new token handled analytically:
      out = (sum_s exp(s)*v_s + e_new*v_new) / (sum_s exp(s) + e_new)
  out_partial = attn @ o_proj[:, own dims].T ; host sums the 8 partials.

KV is trimmed token-exact: request b loads exactly hist[b] cached K columns
(one contiguous DMA, ~4KB/partition runs) and ceil(hist/128) V tiles (the
partial tail tile loads only its valid rows). exp covers only valid score
regions into a zero-initialized probs tile, so no mask tensor and no
denominator fixup. Traffic is spread over three DMA lanes - K on the SP
HWDGE queue, V on the Activation HWDGE queue, weights on the GpSimd SWDGE
queue - with 6-deep KV prefetch so the pipe never starves.
"""

import math
import os

import ml_dtypes
import numpy as np

import concourse.bass as bass
import concourse.mybir as mybir
import concourse.tile as tile
from concourse.bass_utils import run_bass_kernel_spmd
from concourse.vector_clock import ScopedClock

B = 32          # batch (decode requests)
H = 32          # total heads
HL = 4          # heads per core
D = 128         # head dim
HID = 4096
BS = 64         # cache block size
NBLK = 16       # blocks per request
NCORES = 8
KT = HID // 128         # 32 contraction tiles for qkv proj
HD = HL * D             # 512 local attention dims
ROPE_BASE = 10000.0

F32 = mybir.dt.float32
BF = mybir.dt.bfloat16
FP8 = mybir.dt.float8e4
BF_NP = ml_dtypes.bfloat16
FP8_NP = mybir.dt.np(mybir.dt.float8e4)
WKV_SCALE = 64.0   # fp8 scale for the k/v columns of W_pack
EXP_FN = mybir.ActivationFunctionType.Exp
MUL = mybir.AluOpType.mult
ADD = mybir.AluOpType.add

LAST_RESULTS = None  # test harness peeks at this for profiling info

# ---------------------------------------------------------------------------
# This walrus build accepts very few sync-waits per instruction; the Tile
# kernel-tail drain accumulates one wait per sem lane. Split the waits over
# several drain instructions (all before the barrier, so semantics hold).
_MAX_DRAIN_WAITS = 1


def _patched_drain_and_barrier(self, tick_clock, wait_clock):
    nc = self.nc
    drain_inst = nc.sync.drain()
    wait_clock.add_sem_waits(
        drain_inst.ins, ScopedClock({None: tick_clock.global_clock})
    )
    si = drain_inst.ins.sync_info
    if si is not None and si.on_wait and len(si.on_wait) > _MAX_DRAIN_WAITS:
        waits = list(si.on_wait)
        drain_inst.ins.sync_info = mybir.SyncInfo(
            on_wait=waits[:_MAX_DRAIN_WAITS], on_update=list(si.on_update or [])
        )
        rest = waits[_MAX_DRAIN_WAITS:]
        for i in range(0, len(rest), _MAX_DRAIN_WAITS):
            extra = nc.sync.drain()
            extra.ins.sync_info = mybir.SyncInfo(
                on_wait=rest[i : i + _MAX_DRAIN_WAITS], on_update=[]
            )
    nc.all_engine_barrier()
    popped = nc._tile_sem_poison_stack.pop()
    assert popped is self._sem_poison
    nc.clear_and_free_semaphores(list(self.sems.allocated().values()))
    nc.all_engine_barrier()


tile.TileContext._drain_and_barrier = _patched_drain_and_barrier


def _split_excess_waits(nc, limit=1):
    """Walrus rejects instructions carrying more than ~1 sync wait. Hoist the
    excess onto NoOps inserted just before, on the same engine queue (the
    queue blocks on them first, so semantics are identical)."""
    for fn in nc.m.functions:
        for bb in fn.blocks:
            out = []
            changed = False
            for inst in list(bb.instructions):
                si = getattr(inst, "sync_info", None)
                if si is not None and si.on_wait and len(si.on_wait) > limit:
                    waits = list(si.on_wait)
                    extra, keep = waits[:-limit], waits[-limit:]
                    for i in range(0, len(extra), limit):
                        nop = mybir.InstNoOp(
                            name=nc.get_next_instruction_name(),
                            ins=[], outs=[], engine=inst.engine,
                            sync_info=mybir.SyncInfo(
                                on_wait=extra[i : i + limit], on_update=[]
                            ),
                        )
                        nc.register_instruction(nop)
                        out.append(nop)
                    inst.sync_info = mybir.SyncInfo(
                        on_wait=keep, on_update=list(si.on_update or [])
                    )
                    changed = True
                out.append(inst)
            if changed:
                bb.instructions = out
# ---------------------------------------------------------------------------


def _geom(s):
    """s cached tokens -> (full 128-tiles, remainder rows, total tiles)."""
    full = s // 128
    rem = s - 128 * full
    return full, rem, full + (1 if rem else 0)


def _build_nc(svec):
    """Build the SPMD bass module. `svec[b]` = cached tokens for request b
    (same on every core; the head split is via input data)."""
    nc = bass.Bass()

    koff = [0]
    voff = [0]
    for s in svec:
        _, _, pb = _geom(s)
        koff.append(koff[-1] + ((HL * s + 31) // 32) * 32)
        voff.append(voff[-1] + HL * pb * 128)
    NK = max(koff[-1], 1)
    NV = max(voff[-1], 1)

    def param(name, shape, dt):
        return nc.declare_dram_parameter(name, list(shape), dt, isOutput=False)

    hT = param("hT", [128, KT, B], BF)
    wpqk = param("wpqk", [KT, 128, 2 * HD], BF)
    wpv = param("wpv", [KT, 128, HD], FP8)
    wo = param("wo", [HL, 128, HID], BF)
    kc = param("kc", [128, NK], BF)
    vc = param("vc", [128, NV], BF)
    cs = param("cs", [B, 4 * HD], F32)
    identp = param("ident", [B, B], F32)
    out_part = nc.declare_dram_parameter("out_part", [B, HID], BF, isOutput=True)

    with tile.TileContext(nc) as tc:
        with (
            tc.tile_pool(name="const", bufs=1) as cpool,
            tc.tile_pool(name="work", bufs=1) as wpool,
            tc.tile_pool(name="wtiles", bufs=6) as wtp,
            tc.tile_pool(name="wop", bufs=4) as wop,
            tc.tile_pool(name="kv", bufs=7) as kvp,
            tc.tile_pool(name="small", bufs=6) as smp,
        ):
            # ---- constants ----
            ident = cpool.tile([B, B], F32)
            nc.sync.dma_start(out=ident[:], in_=identp[:])
            ones = cpool.tile([128, 1], BF)
            nc.vector.memset(ones[:], 1.0)
            onesf = cpool.tile([1, HL * B], F32)
            nc.vector.memset(onesf[:], 1.0)
            cs_sb = cpool.tile([B, 4 * HD], F32)
            nc.scalar.dma_start(out=cs_sb[:], in_=cs[:])
            hT_sb = cpool.tile([128, KT, B], BF)
            nc.sync.dma_start(out=hT_sb[:], in_=hT[:])

            # per-request KV loads: one contiguous K DMA (sync queue), V in a
            # full-rows DMA plus a partial-tail DMA (scalar queue)
            kv_tiles = {}

            def load_b(b):
                s = svec[b]
                full, rem, pb = _geom(s)
                if pb == 0:
                    kv_tiles[b] = None
                    return
                kcb = kvp.tile([128, HL * s], BF, tag="kc")
                nc.sync.dma_start(
                    out=kcb[:], in_=kc[:, koff[b] : koff[b] + HL * s]
                )
                vcb = kvp.tile([128, HL * pb * 128], BF, tag="vc")
                c1 = HL * full * 128
                if full:
                    nc.scalar.dma_start(
                        out=vcb[:, 0:c1], in_=vc[:, voff[b] : voff[b] + c1]
                    )
                if rem:
                    nc.scalar.dma_start(
                        out=vcb[0:rem, c1 : HL * pb * 128],
                        in_=vc[0:rem, voff[b] + c1 : voff[b] + HL * pb * 128],
                    )
                kv_tiles[b] = (kcb, vcb)

            for b in range(5):
                load_b(b)

            # accumulators written per-b, read in the epilogue
            atsb = wpool.tile([128, HL * B], F32)   # cached attn, col h*32+b
            nc.vector.memset(atsb[:], 0.0)
            dnm = wpool.tile([1, HL * B], F32)      # cached denom, col h*32+b
            nc.vector.memset(dnm[:], 0.0)

            with tc.tile_pool(name="psA", bufs=1, space="PSUM") as psA:
                # PE warmup transpose so `ident` is observed by PE before the
                # real (fp32, single-wait-slot) transposes below.
                tp0 = psA.tile([B, B], F32, tag="tp0")
                nc.tensor.transpose(tp0[:], ident[:], ident[:])

                # ---- phase 1: qkv = hidden @ W_pack (bf16) ----
                qkv_ps = psA.tile([B, 3 * HD], F32, tag="qkv")
                for kt in range(KT):
                    wpqkt = wtp.tile([128, 2 * HD], BF, tag="wpqk")
                    nc.gpsimd.dma_start(out=wpqkt[:], in_=wpqk[kt])
                    wpvt = wtp.tile([128, HD], FP8, tag="wpv")
                    nc.gpsimd.dma_start(out=wpvt[:], in_=wpv[kt])
                    for n in range(2):
                        nc.tensor.matmul(
                            qkv_ps[:, n * HD : (n + 1) * HD],
                            hT_sb[:, kt, :],
                            wpqkt[:, n * HD : (n + 1) * HD],
                            start=(kt == 0),
                            stop=(kt == KT - 1),
                        )
                    nc.tensor.matmul(
                        qkv_ps[:, 2 * HD : 3 * HD], hT_sb[:, kt, :], wpvt[:],
                        start=(kt == 0), stop=(kt == KT - 1),
                    )

                qkv_sb = wpool.tile([B, 3 * HD], F32)
                nc.vector.tensor_copy(qkv_sb[:], qkv_ps[:])

                # ---- phase 2: rotary (fp32, DVE) + transposes ----
                def rope(src_off, cs_off):
                    src = qkv_sb[:, src_off : src_off + HD]
                    t1 = wpool.tile([B, HD], F32, tag="rope_t1")
                    nc.vector.tensor_tensor(
                        t1[:], src, cs_sb[:, cs_off : cs_off + HD], MUL
                    )
                    sh = wpool.tile([B, HD], F32, tag="rope_sh")
                    sh4 = sh[:].rearrange("b (h d) -> b h d", h=HL)
                    sr4 = qkv_sb[:, src_off : src_off + HD].rearrange(
                        "b (h d) -> b h d", h=HL
                    )
                    nc.vector.tensor_copy(sh4[:, :, 0:64], sr4[:, :, 64:128])
                    nc.vector.tensor_copy(sh4[:, :, 64:128], sr4[:, :, 0:64])
                    nc.vector.tensor_tensor(
                        sh[:], sh[:], cs_sb[:, cs_off + HD : cs_off + 2 * HD], MUL
                    )
                    nc.vector.tensor_tensor(
                        qkv_sb[:, src_off : src_off + HD], t1[:], sh[:], ADD
                    )

                rope(0, 0)          # q (scale folded into tables)
                rope(HD, 2 * HD)    # k

                # PE transposes -> [128(d), (h,b)] fp32 tiles
                qT = wpool.tile([128, HL * B], F32)
                kT = wpool.tile([128, HL * B], F32)
                vT = wpool.tile([128, HL * B], F32)
                for off, dst in ((0, qT), (HD, kT), (2 * HD, vT)):
                    for h in range(HL):
                        tp = psA.tile([128, B], F32, tag="tp")
                        inp = qkv_sb[:, off + h * D : off + (h + 1) * D]
                        nc.tensor.transpose(tp[:], inp, ident[:])
                        nc.vector.tensor_copy(dst[:, h * B : (h + 1) * B], tp[:])
                # v came out of the fp8 W_pack columns scaled by WKV_SCALE
                nc.scalar.mul(vT[:], vT[:], 1.0 / WKV_SCALE)

                qT_bf = wpool.tile([128, HL * B], BF)
                nc.vector.tensor_copy(qT_bf[:], qT[:])

                # new-token scores: e_new[(h,b)] = exp(q . k_new)
                prod = wpool.tile([128, HL * B], F32)
                nc.vector.tensor_tensor(prod[:], qT[:], kT[:], MUL)
                prod_bf = wpool.tile([128, HL * B], BF)
                nc.vector.tensor_copy(prod_bf[:], prod[:])
                sn_ps = psA.tile([1, HL * B], F32, tag="sn")
                nc.tensor.matmul(sn_ps[:], ones[:], prod_bf[:], start=True, stop=True)
                e_new = wpool.tile([1, HL * B], F32)
                nc.scalar.activation(e_new[:], sn_ps[:], EXP_FN)

            # ---- phase 3: per-request paged attention ----
            # o_proj weight DMAs are interleaved into the attention tail so
            # they fill the wire without delaying critical-path KV loads
            wo_tiles = {}
            wo_sched = {18: 0, 21: 1, 24: 2, 27: 3}

            def issue_wo(h):
                wot = wop.tile([128, HID], BF, tag="wo")
                nc.gpsimd.dma_start(out=wot[:], in_=wo[h])
                wo_tiles[h] = wot

            with (
                tc.tile_pool(name="psB", bufs=3, space="PSUM") as psB,
                tc.tile_pool(name="psB2", bufs=2, space="PSUM") as psB2,
            ):
                def emit_v(b, probs, vcb, full, rem, pb):
                    # attn^T[d, h] = sum_s p[s] * V[s, d], V stationary
                    atp = psB.tile([128, HL], F32, tag="atp")
                    for h in range(HL):
                        for p in range(pb):
                            w = 128 if p < full else rem
                            col = ((h * full + p) if p < full
                                   else (HL * full + h)) * 128
                            nc.tensor.matmul(
                                atp[:, h : h + 1],
                                vcb[0:w, col : col + 128],
                                probs[0:w, h, p : p + 1],
                                start=(p == 0), stop=(p == pb - 1),
                            )
                    nc.vector.tensor_copy(
                        atsb[:].rearrange("d (h b2) -> d h b2", h=HL)[:, :, b],
                        atp[:],
                    )

                    # denominators: column sums of probs (zeros contribute 0)
                    dsp = psB2.tile([1, HL * pb], F32, tag="dsp")
                    nc.tensor.matmul(
                        dsp[:], ones[:],
                        probs[:].rearrange("s h p -> s (h p)"),
                        start=True, stop=True,
                    )
                    nc.vector.reduce_sum(
                        dnm[:].rearrange("o (h b2) -> o h b2", h=HL)[:, :, b],
                        dsp[:].rearrange("o (h p) -> o h p", h=HL),
                        axis=mybir.AxisListType.X,
                    )

                # natural order, except the big final request is pulled into
                # the middle so the post-last-DMA backlog is small requests
                order = list(range(13)) + [31] + list(range(13, 31))
                for bi, b in enumerate(order):
                    if bi in wo_sched:
                        issue_wo(wo_sched[bi])
                    s = svec[b]
                    full, rem, pb = _geom(s)
                    if pb == 0:
                        continue
                    if b not in kv_tiles:
                        load_b(b)
                    ni = bi + 5
                    while ni < B and svec[order[ni]] == 0:
                        ni += 1
                    if ni < B and order[ni] not in kv_tiles:
                        load_b(order[ni])
                    kcb, vcb = kv_tiles.pop(b)

                    # scores^T: [128(s), (h, pair)], K stationary, q moving
                    scp = psB.tile([128, HL, pb], F32, tag="scp")
                    for h in range(HL):
                        qcol = qT_bf[:, h * B + b : h * B + b + 1]
                        for p in range(pb):
                            w = 128 if p < full else rem
                            nc.tensor.matmul(
                                scp[0:w, h, p : p + 1],
                                kcb[:, h * s + 128 * p : h * s + 128 * p + w],
                                qcol,
                                start=True, stop=True,
                            )

                    # exp of exactly the valid region into zeroed bf16 probs
                    probs = smp.tile([128, HL, pb], BF, tag="probs")
                    if rem:
                        nc.vector.memset(probs[:], 0.0)
                    if full:
                        nc.scalar.activation(
                            probs[:, :, 0:full], scp[:, :, 0:full], EXP_FN
                        )
                    if rem:
                        nc.scalar.activation(
                            probs[0:rem, :, full : full + 1],
                            scp[0:rem, :, full : full + 1],
                            EXP_FN,
                        )

                    emit_v(b, probs, vcb, full, rem, pb)

            # ---- epilogue: add new token, normalize, project ----
            dtot = wpool.tile([1, HL * B], F32)
            nc.vector.tensor_tensor(dtot[:], dnm[:], e_new[:], ADD)
            rec = wpool.tile([1, HL * B], F32)
            nc.vector.reciprocal(rec[:], dtot[:])
            att = wpool.tile([128, HL * B], F32)
            with tc.tile_pool(name="psD", bufs=1, space="PSUM") as psD:
                # broadcast rows across partitions via K=1 outer products
                ebp = psD.tile([128, HL * B], F32, tag="ebp")
                nc.tensor.matmul(ebp[:], onesf[:], e_new[:], start=True, stop=True)
                rbp = psD.tile([128, HL * B], F32, tag="rbp")
                nc.tensor.matmul(rbp[:], onesf[:], rec[:], start=True, stop=True)

                nc.vector.tensor_tensor(att[:], vT[:], ebp[:], MUL)
                nc.vector.tensor_tensor(att[:], att[:], atsb[:], ADD)
                nc.vector.tensor_tensor(att[:], att[:], rbp[:], MUL)
            att_bf = wpool.tile([128, HL * B], BF)
            nc.vector.tensor_copy(att_bf[:], att[:])

            with tc.tile_pool(name="psC", bufs=8, space="PSUM") as psC:
                for h in range(HL):
                    if h not in wo_tiles:
                        issue_wo(h)
                opsn = []
                for _n in range(8):
                    ops_t = psC.tile([B, 512], F32, tag="ops")
                    opsn.append(ops_t)
                for h in range(HL):
                    for n in range(8):
                        nc.tensor.matmul(
                            opsn[n][:],
                            att_bf[:, h * B : (h + 1) * B],
                            wo_tiles[h][:, n * 512 : (n + 1) * 512],
                            start=(h == 0),
                            stop=(h == HL - 1),
                        )
                for n in range(8):
                    outc = smp.tile([B, 512], BF, tag="outc")
                    if n % 2:
                        nc.scalar.copy(outc[:], opsn[n][:])
                    else:
                        nc.vector.tensor_copy(outc[:], opsn[n][:])
                    nc.sync.dma_start(
                        out=out_part[:, n * 512 : (n + 1) * 512], in_=outc[:]
                    )

    _split_excess_waits(nc)
    return nc


def _host_prep(hidden, W_pack, o_proj_weight, k_cache, v_cache, hist, block_offsets):
    """Build the 8 per-core input maps (numpy only)."""
    hidden = np.asarray(hidden, np.float32)
    W_pack = np.asarray(W_pack, np.float32)
    o_proj_weight = np.asarray(o_proj_weight, np.float32)
    k_cache = np.asarray(k_cache, np.float32)
    v_cache = np.asarray(v_cache, np.float32)
    hist = np.asarray(hist, np.int64)
    block_offsets = np.asarray(block_offsets, np.int64)

    svec = [int(x) for x in hist]

    # rope tables, scale folded into the q tables
    inv_freq = 1.0 / (ROPE_BASE ** (np.arange(0, D, 2, dtype=np.float32) / D))
    ang = hist.astype(np.float32)[:, None] * inv_freq[None, :]        # [B, 64]
    cos128 = np.concatenate([np.cos(ang), np.cos(ang)], -1)           # [B, 128]
    sin128 = np.concatenate([np.sin(ang), np.sin(ang)], -1)
    sign = np.concatenate([-np.ones(64), np.ones(64)]).astype(np.float32)
    sc = 1.0 / math.sqrt(D)
    tile_h = lambda x: np.tile(x, (1, HL)).astype(np.float32)         # [B, 512]
    cs = np.concatenate(
        [tile_h(cos128 * sc), tile_h(sin128 * sign * sc),
         tile_h(cos128), tile_h(sin128 * sign)], -1,
    )                                                                 # [B, 2048]

    hT = np.ascontiguousarray(hidden.T).astype(BF_NP)                 # [4096, 32]
    hT = np.ascontiguousarray(hT.reshape(KT, 128, B).transpose(1, 0, 2))

    # gather caches via the block table (b-major), slice heads per core
    k_all = k_cache[block_offsets.reshape(-1)].reshape(B, NBLK * BS, H, D)
    v_all = v_cache[block_offsets.reshape(-1)].reshape(B, NBLK * BS, H, D)

    ident = np.eye(B, dtype=np.float32)

    in_maps = []
    for c in range(NCORES):
        h0 = c * HL
        qcols = np.arange(h0 * D, (h0 + HL) * D)
        wpqk_c = np.concatenate(
            [W_pack[:, qcols], W_pack[:, HID + qcols]], axis=1,
        ).astype(BF_NP).reshape(KT, 128, 2 * HD)
        wpv_c = (W_pack[:, 2 * HID + qcols] * WKV_SCALE).astype(
            FP8_NP).reshape(KT, 128, HD)

        wo_c = np.ascontiguousarray(o_proj_weight[:, qcols].T).astype(BF_NP)
        wo_c = wo_c.reshape(HL, 128, HID)                             # [512, 4096]

        # K: [128(d), concat_b(h-major: s_b cols)] token-exact
        # V: [128(s in tile), concat_b(full tiles h-major, then partial per h)]
        kcols = []
        vcols = []
        for b in range(B):
            s = svec[b]
            full, rem, pb = _geom(s)
            if pb == 0:
                continue
            kb = k_all[b, :s, h0 : h0 + HL, :]                        # [s, 4, 128]
            for h in range(HL):
                kcols.append(np.ascontiguousarray(kb[:, h, :].T))     # [128, s]
            pad = ((HL * s + 31) // 32) * 32 - HL * s
            if pad:
                kcols.append(np.zeros((128, pad), np.float32))
            vb = v_all[b, :s, h0 : h0 + HL, :]                        # [s, 4, 128]
            for h in range(HL):
                for p in range(full):
                    vcols.append(vb[128 * p : 128 * (p + 1), h, :])   # [128, 128]
            if rem:
                for h in range(HL):
                    t = np.zeros((128, D), np.float32)
                    t[:rem] = vb[128 * full :, h, :]
                    vcols.append(t)
        kc = (np.concatenate(kcols, axis=1) if kcols
              else np.zeros((128, 1), np.float32)).astype(BF_NP)
        vc = (np.concatenate(vcols, axis=1) if vcols
              else np.zeros((128, 1), np.float32)).astype(BF_NP)

        in_maps.append({
            "hT": hT, "wpqk": wpqk_c, "wpv": wpv_c, "wo": wo_c,
            "kc": np.ascontiguousarray(kc), "vc": np.ascontiguousarray(vc),
            "cs": cs, "ident": ident,
        })
    return svec, in_maps


def kernel(hidden_states, W_pack, o_proj_weight, k_cache, v_cache,
           history_lengths, block_offsets):
    global LAST_RESULTS
    svec, in_maps = _host_prep(
        hidden_states, W_pack, o_proj_weight, k_cache, v_cache,
        history_lengths, block_offsets,
    )
    nc = _build_nc(svec)
    trace = bool(int(os.environ.get("KERNEL_TRACE", "0")))
    res = run_bass_kernel_spmd(nc, in_maps, list(range(NCORES)), trace=trace)
    LAST_RESULTS = res
    out = np.zeros((B, HID), np.float32)
    for c in range(NCORES):
        out += res.results[c]["out_part"]
    return out


# revision 53
# speedup vs baseline: 1.0709x; 1.0709x over previous
"""Paged decode attention (nn_Attention_5626407157951) on 8 Trainium2 cores.

Tensor-parallel over heads: each core owns 4 of 32 heads. The kernel is
HBM-bound (per-core DMA tops out ~250 GB/s regardless of queue count), so
everything is sized to minimize bytes: single-term bf16 matmuls everywhere
(vs hi/lo fp32 emulation: 3x less PE work and 33% less traffic), and the
v columns of W_pack in fp8*64 (v_new feeds one token of ~500; measured
total error 9e-3 vs the 2e-2 gate). Per core:
  qkv = hidden @ W_pack[:, own cols]      (bf16 / fp8, fp32 acc in PSUM)
  rotary(q, k) at pos=hist                (DVE, fp32; host-built cos/sin)
  scores_T[s, (h,p)] = K_cache^T q        (PE, K stationary, q moving 1 col)
  softmax without max-subtraction; new token handled analytically:
      out = (sum_s exp(s)*v_s + e_new*v_new) / (sum_s exp(s) + e_new)
  out_partial = attn @ o_proj[:, own dims].T ; host sums the 8 partials.

KV is trimmed token-exact: request b loads exactly hist[b] cached K columns
(one contiguous DMA, ~4KB/partition runs) and ceil(hist/128) V tiles (the
partial tail tile loads only its valid rows). exp covers only valid score
regions into a zero-initialized probs tile, so no mask tensor and no
denominator fixup. Traffic is spread over three DMA lanes - K on the SP
HWDGE queue, V on the Activation HWDGE queue, weights on the GpSimd SWDGE
queue - with 6-deep KV prefetch so the pipe never starves.
"""

import math
import os

import ml_dtypes
import numpy as np

import concourse.bass as bass
import concourse.mybir as mybir
import concourse.tile as tile
from concourse.bass_utils import run_bass_kernel_spmd
from concourse.vector_clock import ScopedClock

B = 32          # batch (decode requests)
H = 32          # total heads
HL = 4          # heads per core
D = 128         # head dim
HID = 4096
BS = 64         # cache block size
NBLK = 16       # blocks per request
NCORES = 8
KT = HID // 128         # 32 contraction tiles for qkv proj
HD = HL * D             # 512 local attention dims
ROPE_BASE = 10000.0

F32 = mybir.dt.float32
BF = mybir.dt.bfloat16
FP8 = mybir.dt.float8e4
BF_NP = ml_dtypes.bfloat16
FP8_NP = mybir.dt.np(mybir.dt.float8e4)
WKV_SCALE = 64.0   # fp8 scale for the k/v columns of W_pack
EXP_FN = mybir.ActivationFunctionType.Exp
MUL = mybir.AluOpType.mult
ADD = mybir.AluOpType.add

LAST_RESULTS = None  # test harness peeks at this for profiling info

# ---------------------------------------------------------------------------
# This walrus build accepts very few sync-waits per instruction; the Tile
# kernel-tail drain accumulates one wait per sem lane. Split the waits over
# several drain instructions (all before the barrier, so semantics hold).
_MAX_DRAIN_WAITS = 1


def _patched_drain_and_barrier(self, tick_clock, wait_clock):
    nc = self.nc
    drain_inst = nc.sync.drain()
    wait_clock.add_sem_waits(
        drain_inst.ins, ScopedClock({None: tick_clock.global_clock})
    )
    si = drain_inst.ins.sync_info
    if si is not None and si.on_wait and len(si.on_wait) > _MAX_DRAIN_WAITS:
        waits = list(si.on_wait)
        drain_inst.ins.sync_info = mybir.SyncInfo(
            on_wait=waits[:_MAX_DRAIN_WAITS], on_update=list(si.on_update or [])
        )
        rest = waits[_MAX_DRAIN_WAITS:]
        for i in range(0, len(rest), _MAX_DRAIN_WAITS):
            extra = nc.sync.drain()
            extra.ins.sync_info = mybir.SyncInfo(
                on_wait=rest[i : i + _MAX_DRAIN_WAITS], on_update=[]
            )
    nc.all_engine_barrier()
    popped = nc._tile_sem_poison_stack.pop()
    assert popped is self._sem_poison
    nc.clear_and_free_semaphores(list(self.sems.allocated().values()))
    nc.all_engine_barrier()


tile.TileContext._drain_and_barrier = _patched_drain_and_barrier


def _split_excess_waits(nc, limit=1):
    """Walrus rejects instructions carrying more than ~1 sync wait. Hoist the
    excess onto NoOps inserted just before, on the same engine queue (the
    queue blocks on them first, so semantics are identical)."""
    for fn in nc.m.functions:
        for bb in fn.blocks:
            out = []
            changed = False
            for inst in list(bb.instructions):
                si = getattr(inst, "sync_info", None)
                if si is not None and si.on_wait and len(si.on_wait) > limit:
                    waits = list(si.on_wait)
                    extra, keep = waits[:-limit], waits[-limit:]
                    for i in range(0, len(extra), limit):
                        nop = mybir.InstNoOp(
                            name=nc.get_next_instruction_name(),
                            ins=[], outs=[], engine=inst.engine,
                            sync_info=mybir.SyncInfo(
                                on_wait=extra[i : i + limit], on_update=[]
                            ),
                        )
                        nc.register_instruction(nop)
                        out.append(nop)
                    inst.sync_info = mybir.SyncInfo(
                        on_wait=keep, on_update=list(si.on_update or [])
                    )
                    changed = True
                out.append(inst)
            if changed:
                bb.instructions = out
# ---------------------------------------------------------------------------


def _geom(s):
    """s cached tokens -> (full 128-tiles, remainder rows, total tiles)."""
    full = s // 128
    rem = s - 128 * full
    return full, rem, full + (1 if rem else 0)


def _build_nc(svec):
    """Build the SPMD bass module. `svec[b]` = cached tokens for request b
    (same on every core; the head split is via input data)."""
    nc = bass.Bass()

    koff = [0]
    voff = [0]
    for s in svec:
        _, _, pb = _geom(s)
        koff.append(koff[-1] + HL * s)
        voff.append(voff[-1] + HL * pb * 128)
    NK = max(koff[-1], 1)
    NV = max(voff[-1], 1)

    def param(name, shape, dt):
        return nc.declare_dram_parameter(name, list(shape), dt, isOutput=False)

    hT = param("hT", [128, KT, B], BF)
    wpqk = param("wpqk", [KT, 128, 2 * HD], BF)
    wpv = param("wpv", [KT, 128, HD], FP8)
    wo = param("wo", [HL, 128, HID], BF)
    kc = param("kc", [128, NK], BF)
    vc = param("vc", [128, NV], BF)
    cs = param("cs", [B, 4 * HD], F32)
    identp = param("ident", [B, B], F32)
    out_part = nc.declare_dram_parameter("out_part", [B, HID], BF, isOutput=True)

    with tile.TileContext(nc) as tc:
        with (
            tc.tile_pool(name="const", bufs=1) as cpool,
            tc.tile_pool(name="work", bufs=1) as wpool,
            tc.tile_pool(name="wtiles", bufs=6) as wtp,
            tc.tile_pool(name="wop", bufs=4) as wop,
            tc.tile_pool(name="kv", bufs=7) as kvp,
            tc.tile_pool(name="small", bufs=6) as smp,
        ):
            # ---- constants ----
            ident = cpool.tile([B, B], F32)
            nc.sync.dma_start(out=ident[:], in_=identp[:])
            ones = cpool.tile([128, 1], BF)
            nc.vector.memset(ones[:], 1.0)
            onesf = cpool.tile([1, HL * B], F32)
            nc.vector.memset(onesf[:], 1.0)
            cs_sb = cpool.tile([B, 4 * HD], F32)
            nc.scalar.dma_start(out=cs_sb[:], in_=cs[:])
            hT_sb = cpool.tile([128, KT, B], BF)
            nc.sync.dma_start(out=hT_sb[:], in_=hT[:])

            # per-request KV loads: one contiguous K DMA (sync queue), V in a
            # full-rows DMA plus a partial-tail DMA (scalar queue)
            kv_tiles = {}

            def load_b(b):
                s = svec[b]
                full, rem, pb = _geom(s)
                if pb == 0:
                    kv_tiles[b] = None
                    return
                kcb = kvp.tile([128, HL * s], BF, tag="kc")
                nc.sync.dma_start(
                    out=kcb[:], in_=kc[:, koff[b] : koff[b] + HL * s]
                )
                vcb = kvp.tile([128, HL * pb * 128], BF, tag="vc")
                c1 = HL * full * 128
                if full:
                    nc.scalar.dma_start(
                        out=vcb[:, 0:c1], in_=vc[:, voff[b] : voff[b] + c1]
                    )
                if rem:
                    nc.scalar.dma_start(
                        out=vcb[0:rem, c1 : HL * pb * 128],
                        in_=vc[0:rem, voff[b] + c1 : voff[b] + HL * pb * 128],
                    )
                kv_tiles[b] = (kcb, vcb)

            for b in range(5):
                load_b(b)

            # accumulators written per-b, read in the epilogue
            atsb = wpool.tile([128, HL * B], F32)   # cached attn, col h*32+b
            nc.vector.memset(atsb[:], 0.0)
            dnm = wpool.tile([1, HL * B], F32)      # cached denom, col h*32+b
            nc.vector.memset(dnm[:], 0.0)

            with tc.tile_pool(name="psA", bufs=1, space="PSUM") as psA:
                # PE warmup transpose so `ident` is observed by PE before the
                # real (fp32, single-wait-slot) transposes below.
                tp0 = psA.tile([B, B], F32, tag="tp0")
                nc.tensor.transpose(tp0[:], ident[:], ident[:])

                # ---- phase 1: qkv = hidden @ W_pack (bf16) ----
                qkv_ps = psA.tile([B, 3 * HD], F32, tag="qkv")
                for kt in range(KT):
                    wpqkt = wtp.tile([128, 2 * HD], BF, tag="wpqk")
                    nc.gpsimd.dma_start(out=wpqkt[:], in_=wpqk[kt])
                    wpvt = wtp.tile([128, HD], FP8, tag="wpv")
                    nc.gpsimd.dma_start(out=wpvt[:], in_=wpv[kt])
                    for n in range(2):
                        nc.tensor.matmul(
                            qkv_ps[:, n * HD : (n + 1) * HD],
                            hT_sb[:, kt, :],
                            wpqkt[:, n * HD : (n + 1) * HD],
                            start=(kt == 0),
                            stop=(kt == KT - 1),
                        )
                    nc.tensor.matmul(
                        qkv_ps[:, 2 * HD : 3 * HD], hT_sb[:, kt, :], wpvt[:],
                        start=(kt == 0), stop=(kt == KT - 1),
                    )

                qkv_sb = wpool.tile([B, 3 * HD], F32)
                nc.vector.tensor_copy(qkv_sb[:], qkv_ps[:])

                # ---- phase 2: rotary (fp32, DVE) + transposes ----
                def rope(src_off, cs_off):
                    src = qkv_sb[:, src_off : src_off + HD]
                    t1 = wpool.tile([B, HD], F32, tag="rope_t1")
                    nc.vector.tensor_tensor(
                        t1[:], src, cs_sb[:, cs_off : cs_off + HD], MUL
                    )
                    sh = wpool.tile([B, HD], F32, tag="rope_sh")
                    sh4 = sh[:].rearrange("b (h d) -> b h d", h=HL)
                    sr4 = qkv_sb[:, src_off : src_off + HD].rearrange(
                        "b (h d) -> b h d", h=HL
                    )
                    nc.vector.tensor_copy(sh4[:, :, 0:64], sr4[:, :, 64:128])
                    nc.vector.tensor_copy(sh4[:, :, 64:128], sr4[:, :, 0:64])
                    nc.vector.tensor_tensor(
                        sh[:], sh[:], cs_sb[:, cs_off + HD : cs_off + 2 * HD], MUL
                    )
                    nc.vector.tensor_tensor(
                        qkv_sb[:, src_off : src_off + HD], t1[:], sh[:], ADD
                    )

                rope(0, 0)          # q (scale folded into tables)
                rope(HD, 2 * HD)    # k

                # PE transposes -> [128(d), (h,b)] fp32 tiles
                qT = wpool.tile([128, HL * B], F32)
                kT = wpool.tile([128, HL * B], F32)
                vT = wpool.tile([128, HL * B], F32)
                for off, dst in ((0, qT), (HD, kT), (2 * HD, vT)):
                    for h in range(HL):
                        tp = psA.tile([128, B], F32, tag="tp")
                        inp = qkv_sb[:, off + h * D : off + (h + 1) * D]
                        nc.tensor.transpose(tp[:], inp, ident[:])
                        nc.vector.tensor_copy(dst[:, h * B : (h + 1) * B], tp[:])
                # v came out of the fp8 W_pack columns scaled by WKV_SCALE
                nc.scalar.mul(vT[:], vT[:], 1.0 / WKV_SCALE)

                qT_bf = wpool.tile([128, HL * B], BF)
                nc.vector.tensor_copy(qT_bf[:], qT[:])

                # new-token scores: e_new[(h,b)] = exp(q . k_new)
                prod = wpool.tile([128, HL * B], F32)
                nc.vector.tensor_tensor(prod[:], qT[:], kT[:], MUL)
                prod_bf = wpool.tile([128, HL * B], BF)
                nc.vector.tensor_copy(prod_bf[:], prod[:])
                sn_ps = psA.tile([1, HL * B], F32, tag="sn")
                nc.tensor.matmul(sn_ps[:], ones[:], prod_bf[:], start=True, stop=True)
                e_new = wpool.tile([1, HL * B], F32)
                nc.scalar.activation(e_new[:], sn_ps[:], EXP_FN)

            # ---- phase 3: per-request paged attention ----
            # o_proj weight DMAs are interleaved into the attention tail so
            # they fill the wire without delaying critical-path KV loads
            wo_tiles = {}
            wo_sched = {18: 0, 21: 1, 24: 2, 27: 3}

            def issue_wo(h):
                wot = wop.tile([128, HID], BF, tag="wo")
                nc.gpsimd.dma_start(out=wot[:], in_=wo[h])
                wo_tiles[h] = wot

            with (
                tc.tile_pool(name="psB", bufs=3, space="PSUM") as psB,
                tc.tile_pool(name="psB2", bufs=2, space="PSUM") as psB2,
            ):
                def emit_v(b, probs, vcb, full, rem, pb):
                    # attn^T[d, h] = sum_s p[s] * V[s, d], V stationary
                    atp = psB.tile([128, HL], F32, tag="atp")
                    for h in range(HL):
                        for p in range(pb):
                            w = 128 if p < full else rem
                            col = ((h * full + p) if p < full
                                   else (HL * full + h)) * 128
                            nc.tensor.matmul(
                                atp[:, h : h + 1],
                                vcb[0:w, col : col + 128],
                                probs[0:w, h, p : p + 1],
                                start=(p == 0), stop=(p == pb - 1),
                            )
                    nc.vector.tensor_copy(
                        atsb[:].rearrange("d (h b2) -> d h b2", h=HL)[:, :, b],
                        atp[:],
                    )

                    # denominators: column sums of probs (zeros contribute 0)
                    dsp = psB2.tile([1, HL * pb], F32, tag="dsp")
                    nc.tensor.matmul(
                        dsp[:], ones[:],
                        probs[:].rearrange("s h p -> s (h p)"),
                        start=True, stop=True,
                    )
                    nc.vector.reduce_sum(
                        dnm[:].rearrange("o (h b2) -> o h b2", h=HL)[:, :, b],
                        dsp[:].rearrange("o (h p) -> o h p", h=HL),
                        axis=mybir.AxisListType.X,
                    )

                # natural order, except the big final request is pulled into
                # the middle so the post-last-DMA backlog is small requests
                order = list(range(13)) + [31] + list(range(13, 31))
                for bi, b in enumerate(order):
                    if bi in wo_sched:
                        issue_wo(wo_sched[bi])
                    s = svec[b]
                    full, rem, pb = _geom(s)
                    if pb == 0:
                        continue
                    if b not in kv_tiles:
                        load_b(b)
                    ni = bi + 5
                    while ni < B and svec[order[ni]] == 0:
                        ni += 1
                    if ni < B and order[ni] not in kv_tiles:
                        load_b(order[ni])
                    kcb, vcb = kv_tiles.pop(b)

                    # scores^T: [128(s), (h, pair)], K stationary, q moving
                    scp = psB.tile([128, HL, pb], F32, tag="scp")
                    for h in range(HL):
                        qcol = qT_bf[:, h * B + b : h * B + b + 1]
                        for p in range(pb):
                            w = 128 if p < full else rem
                            nc.tensor.matmul(
                                scp[0:w, h, p : p + 1],
                                kcb[:, h * s + 128 * p : h * s + 128 * p + w],
                                qcol,
                                start=True, stop=True,
                            )

                    # exp of exactly the valid region into zeroed bf16 probs
                    probs = smp.tile([128, HL, pb], BF, tag="probs")
                    if rem:
                        nc.vector.memset(probs[:], 0.0)
                    if full:
                        nc.scalar.activation(
                            probs[:, :, 0:full], scp[:, :, 0:full], EXP_FN
                        )
                    if rem:
                        nc.scalar.activation(
                            probs[0:rem, :, full : full + 1],
                            scp[0:rem, :, full : full + 1],
                            EXP_FN,
                        )

                    emit_v(b, probs, vcb, full, rem, pb)

            # ---- epilogue: add new token, normalize, project ----
            dtot = wpool.tile([1, HL * B], F32)
            nc.vector.tensor_tensor(dtot[:], dnm[:], e_new[:], ADD)
            rec = wpool.tile([1, HL * B], F32)
            nc.vector.reciprocal(rec[:], dtot[:])
            att = wpool.tile([128, HL * B], F32)
            with tc.tile_pool(name="psD", bufs=1, space="PSUM") as psD:
                # broadcast rows across partitions via K=1 outer products
                ebp = psD.tile([128, HL * B], F32, tag="ebp")
                nc.tensor.matmul(ebp[:], onesf[:], e_new[:], start=True, stop=True)
                rbp = psD.tile([128, HL * B], F32, tag="rbp")
                nc.tensor.matmul(rbp[:], onesf[:], rec[:], start=True, stop=True)

                nc.vector.tensor_tensor(att[:], vT[:], ebp[:], MUL)
                nc.vector.tensor_tensor(att[:], att[:], atsb[:], ADD)
                nc.vector.tensor_tensor(att[:], att[:], rbp[:], MUL)
            att_bf = wpool.tile([128, HL * B], BF)
            nc.vector.tensor_copy(att_bf[:], att[:])

            with tc.tile_pool(name="psC", bufs=8, space="PSUM") as psC:
                for h in range(HL):
                    if h not in wo_tiles:
                        issue_wo(h)
                opsn = []
                for _n in range(8):
                    ops_t = psC.tile([B, 512], F32, tag="ops")
                    opsn.append(ops_t)
                for h in range(HL):
                    for n in range(8):
                        nc.tensor.matmul(
                            opsn[n][:],
                            att_bf[:, h * B : (h + 1) * B],
                            wo_tiles[h][:, n * 512 : (n + 1) * 512],
                            start=(h == 0),
                            stop=(h == HL - 1),
                        )
                for n in range(8):
                    outc = smp.tile([B, 512], BF, tag="outc")
                    if n % 2:
                        nc.scalar.copy(outc[:], opsn[n][:])
                    else:
                        nc.vector.tensor_copy(outc[:], opsn[n][:])
                    nc.sync.dma_start(
                        out=out_part[:, n * 512 : (n + 1) * 512], in_=outc[:]
                    )

    _split_excess_waits(nc)
    return nc


def _host_prep(hidden, W_pack, o_proj_weight, k_cache, v_cache, hist, block_offsets):
    """Build the 8 per-core input maps (numpy only)."""
    hidden = np.asarray(hidden, np.float32)
    W_pack = np.asarray(W_pack, np.float32)
    o_proj_weight = np.asarray(o_proj_weight, np.float32)
    k_cache = np.asarray(k_cache, np.float32)
    v_cache = np.asarray(v_cache, np.float32)
    hist = np.asarray(hist, np.int64)
    block_offsets = np.asarray(block_offsets, np.int64)

    svec = [int(x) for x in hist]

    # rope tables, scale folded into the q tables
    inv_freq = 1.0 / (ROPE_BASE ** (np.arange(0, D, 2, dtype=np.float32) / D))
    ang = hist.astype(np.float32)[:, None] * inv_freq[None, :]        # [B, 64]
    cos128 = np.concatenate([np.cos(ang), np.cos(ang)], -1)           # [B, 128]
    sin128 = np.concatenate([np.sin(ang), np.sin(ang)], -1)
    sign = np.concatenate([-np.ones(64), np.ones(64)]).astype(np.float32)
    sc = 1.0 / math.sqrt(D)
    tile_h = lambda x: np.tile(x, (1, HL)).astype(np.float32)         # [B, 512]
    cs = np.concatenate(
        [tile_h(cos128 * sc), tile_h(sin128 * sign * sc),
         tile_h(cos128), tile_h(sin128 * sign)], -1,
    )                                                                 # [B, 2048]

    hT = np.ascontiguousarray(hidden.T).astype(BF_NP)                 # [4096, 32]
    hT = np.ascontiguousarray(hT.reshape(KT, 128, B).transpose(1, 0, 2))

    # gather caches via the block table (b-major), slice heads per core
    k_all = k_cache[block_offsets.reshape(-1)].reshape(B, NBLK * BS, H, D)
    v_all = v_cache[block_offsets.reshape(-1)].reshape(B, NBLK * BS, H, D)

    ident = np.eye(B, dtype=np.float32)

    in_maps = []
    for c in range(NCORES):
        h0 = c * HL
        qcols = np.arange(h0 * D, (h0 + HL) * D)
        wpqk_c = np.concatenate(
            [W_pack[:, qcols], W_pack[:, HID + qcols]], axis=1,
        ).astype(BF_NP).reshape(KT, 128, 2 * HD)
        wpv_c = (W_pack[:, 2 * HID + qcols] * WKV_SCALE).astype(
            FP8_NP).reshape(KT, 128, HD)

        wo_c = np.ascontiguousarray(o_proj_weight[:, qcols].T).astype(BF_NP)
        wo_c = wo_c.reshape(HL, 128, HID)                             # [512, 4096]

        # K: [128(d), concat_b(h-major: s_b cols)] token-exact
        # V: [128(s in tile), concat_b(full tiles h-major, then partial per h)]
        kcols = []
        vcols = []
        for b in range(B):
            s = svec[b]
            full, rem, pb = _geom(s)
            if pb == 0:
                continue
            kb = k_all[b, :s, h0 : h0 + HL, :]                        # [s, 4, 128]
            for h in range(HL):
                kcols.append(np.ascontiguousarray(kb[:, h, :].T))     # [128, s]
            vb = v_all[b, :s, h0 : h0 + HL, :]                        # [s, 4, 128]
            for h in range(HL):
                for p in range(full):
                    vcols.append(vb[128 * p : 128 * (p + 1), h, :])   # [128, 128]
            if rem:
                for h in range(HL):
                    t = np.zeros((128, D), np.float32)
                    t[:rem] = vb[128 * full :, h, :]
                    vcols.append(t)
        kc = (np.concatenate(kcols, axis=1) if kcols
              else np.zeros((128, 1), np.float32)).astype(BF_NP)
        vc = (np.concatenate(vcols, axis=1) if vcols
              else np.zeros((128, 1), np.float32)).astype(BF_NP)

        in_maps.append({
            "hT": hT, "wpqk": wpqk_c, "wpv": wpv_c, "wo": wo_c,
            "kc": np.ascontiguousarray(kc), "vc": np.ascontiguousarray(vc),
            "cs": cs, "ident": ident,
        })
    return svec, in_maps


def kernel(hidden_states, W_pack, o_proj_weight, k_cache, v_cache,
           history_lengths, block_offsets):
    global LAST_RESULTS
    svec, in_maps = _host_prep(
        hidden_states, W_pack, o_proj_weight, k_cache, v_cache,
        history_lengths, block_offsets,
    )
    nc = _build_nc(svec)
    trace = bool(int(os.environ.get("KERNEL_TRACE", "0")))
    res = run_bass_kernel_spmd(nc, in_maps, list(range(NCORES)), trace=trace)
    LAST_RESULTS = res
    out = np.zeros((B, HID), np.float32)
    for c in range(NCORES):
        out += res.results[c]["out_part"]
    return out
